# revision 15
# baseline (speedup 1.0000x reference)
"""Trainium2 Bass kernel for nn_ClassifierAttn (single-query attention pooling).

Math restructuring (exact, up to float assoc):
  reference computes, per (b,q):
    v = e @ Wvp^T + bvp            [S, H]      (e = video_enc[b,q], S=2048)
    k = v @ Wk^T ; val = v @ Wv^T
    scores = k . qq / 32 ; attn = softmax(scores)
    pooled = attn @ val
  Since the attention query is a single vector per (b,q):
    scores[s] = (e[s] . cv + const)/32   with cv = Wvp^T (Wk^T qq),
  and the const term cancels in softmax. Similarly
    pooled = ((attn @ e) @ Wvp^T + bvp) @ Wv^T.
  So the big tensor e is touched by exactly two mat-vec passes:
    (1) scores = e . cv  (fused mul+reduce on VectorE)
    (2) ebar = sum_s p[s] e[s,:]  (PE matmul, p = unnormalized softmax weights)
  Everything else is tiny [10, *] linear algebra.

Sharding: (B*QPV)=80 pairs, 10 per core across 8 cores; weights replicated.
"""

import os
import numpy as np

P = 128
NBQ = 10          # (b,q) pairs per core
SO = 16           # S / P
S = 2048
D = 768
H = 512
H2 = 1024
OUT = 5
N_CORES = 8
INV_SQRT = 1.0 / 32.0   # 1/sqrt(2H)

_COMPILED = None
LAST_RESULTS = None


def ts(i, size):
    return slice(i * size, (i + 1) * size)


def _build():
    import concourse.bass as bass  # noqa: F401
    import concourse.mybir as mybir
    import concourse.tile as tile
    from concourse import bacc
    from concourse.masks import make_identity

    fp32 = mybir.dt.float32
    f32r = mybir.dt.float32r
    Alu = mybir.AluOpType
    Act = mybir.ActivationFunctionType
    Ax = mybir.AxisListType

    nc = bacc.Bacc("TRN2", target_bir_lowering=False, debug=False,
                   num_devices=N_CORES)

    # ---- DRAM I/O (all host-pre-tiled to DMA-contiguous layouts) ----
    ve = nc.dram_tensor("ve", [NBQ, SO, P, D], f32r, kind="ExternalInput")
    q0T = nc.dram_tensor("q0T", [P, 6, NBQ], fp32, kind="ExternalInput")
    omask = nc.dram_tensor("omask", [NBQ, OUT], fp32, kind="ExternalInput")
    WqpT = nc.dram_tensor("WqpT", [P, 6, H], fp32, kind="ExternalInput")
    WqT = nc.dram_tensor("WqT", [P, 4, H2], fp32, kind="ExternalInput")
    Wk = nc.dram_tensor("Wk", [P, 8, H], fp32, kind="ExternalInput")
    Wvp = nc.dram_tensor("Wvp", [P, 4, D], fp32, kind="ExternalInput")
    WvpT = nc.dram_tensor("WvpT", [P, 6, H], fp32, kind="ExternalInput")
    WvT = nc.dram_tensor("WvT", [P, 4, H], fp32, kind="ExternalInput")
    W1T = nc.dram_tensor("W1T", [P, 8, H], fp32, kind="ExternalInput")
    W2T = nc.dram_tensor("W2T", [P, 4, OUT], fp32, kind="ExternalInput")
    bqp = nc.dram_tensor("bqp", [P, 4], fp32, kind="ExternalInput")
    bvp = nc.dram_tensor("bvp", [P, 4], fp32, kind="ExternalInput")
    b1 = nc.dram_tensor("b1", [P, 4], fp32, kind="ExternalInput")
    b2 = nc.dram_tensor("b2", [1, OUT], fp32, kind="ExternalInput")
    out = nc.dram_tensor("out", [NBQ, OUT], fp32, kind="ExternalOutput")

    with tile.TileContext(nc) as tc:
        with (
            tc.tile_pool(name="const", bufs=1) as cw,
            tc.tile_pool(name="stream", bufs=2) as st,
            tc.tile_pool(name="small", bufs=2) as sm,
            tc.tile_pool(name="ps_small", bufs=2, space="PSUM") as ps_small,
            tc.tile_pool(name="ps_rowA", bufs=2, space="PSUM") as ps_rowA,
            tc.tile_pool(name="ps_rowB", bufs=2, space="PSUM") as ps_rowB,
            tc.tile_pool(name="ps_z", bufs=2, space="PSUM") as ps_z,
        ):
            # ---- load weights / small inputs ----
            def mkload(pool):
                def load(dram, shape):
                    t = pool.tile(shape, fp32, tag=dram.name + "_sb")
                    nc.sync.dma_start(t[:], dram.ap())
                    return t
                return load

            load = mkload(cw)
            wvpT = load(WvpT, [P, 6, H])
            wvT = load(WvT, [P, 4, H])
            w1T = load(W1T, [P, 8, H])
            w2T = load(W2T, [P, 4, OUT])
            sb_bqp = load(bqp, [P, 4])
            sb_bvp = load(bvp, [P, 4])
            sb_b1 = load(b1, [P, 4])
            sb_q0T = load(q0T, [P, 6, NBQ])
            sb_om = load(omask, [NBQ, OUT])

            sb_b2b = cw.tile([NBQ, OUT], fp32, tag="b2b")
            nc.sync.dma_start(sb_b2b[:], b2.ap().to_broadcast((NBQ, OUT)))
            sb_mb = cw.tile([NBQ, OUT], fp32, tag="mb")
            nc.vector.tensor_add(sb_mb[:], sb_om[:], sb_b2b[:])

            ones_col = cw.tile([P, 1], fp32, tag="ones")
            nc.gpsimd.memset(ones_col[:], 1.0)
            ones_r = cw.tile([P, 1], f32r, tag="ones_r")
            nc.vector.tensor_copy(ones_r[:], ones_col[:])
            id10 = cw.tile([NBQ, NBQ], fp32, tag="id10")
            make_identity(nc, id10[:])

            # ---- prologue: q path -> cv_rows [10, 768] ----
            # prologue-only weights live in their own pool, freed before the
            # big streaming tiles are allocated
            qT = cw.tile([P, 4, NBQ], fp32, tag="qT")
            cv_rows = cw.tile([NBQ, D], fp32, tag="cv_rows")
            with tc.tile_pool(name="prologue_w", bufs=2) as pw:
                def loadp(dram, shape):
                    t = pw.tile(shape, fp32, tag="wpro")
                    nc.sync.dma_start(t[:], dram.ap())
                    return t

                wqpT = loadp(WqpT, [P, 6, H])
                for mc in range(4):
                    pq = ps_small.tile([P, NBQ], fp32, tag="ps")
                    for kc in range(6):
                        nc.tensor.matmul(pq[:], wqpT[:, kc, ts(mc, P)],
                                         sb_q0T[:, kc, :],
                                         start=(kc == 0), stop=(kc == 5))
                    nc.vector.tensor_scalar(qT[:, mc, :], pq[:],
                                            sb_bqp[:, mc:mc + 1], None, Alu.add)

                wqT = loadp(WqT, [P, 4, H2])
                qqT = cw.tile([P, 8, NBQ], fp32, tag="qqT")
                for mc in range(8):
                    pq = ps_small.tile([P, NBQ], fp32, tag="ps")
                    for kc in range(4):
                        nc.tensor.matmul(pq[:], wqT[:, kc, ts(mc, P)],
                                         qT[:, kc, :],
                                         start=(kc == 0), stop=(kc == 3))
                    nc.scalar.copy(qqT[:, mc, :], pq[:])

                wk = loadp(Wk, [P, 8, H])
                ckT = cw.tile([P, 4, NBQ], fp32, tag="ckT")
                for mc in range(4):
                    pq = ps_small.tile([P, NBQ], fp32, tag="ps")
                    for kc in range(8):
                        nc.tensor.matmul(pq[:], wk[:, kc, ts(mc, P)],
                                         qqT[:, kc, :],
                                         start=(kc == 0), stop=(kc == 7))
                    nc.scalar.copy(ckT[:, mc, :], pq[:])

                wvp = loadp(Wvp, [P, 4, D])
                for nsl, nsz in ((slice(0, 512), 512), (slice(512, 768), 256)):
                    pq = ps_small.tile([NBQ, 512], fp32, tag="ps")
                    for kc in range(4):
                        nc.tensor.matmul(pq[:, :nsz], ckT[:, kc, :],
                                         wvp[:, kc, nsl],
                                         start=(kc == 0), stop=(kc == 3))
                    nc.scalar.copy(cv_rows[:, nsl], pq[:, :nsz])

            # ---- streaming phase over the 10 (b,q) pairs ----
            ebar_rows = cw.tile([NBQ, D], fp32, tag="ebar_rows")
            for i in range(NBQ):
                cv_stage = sm.tile([1, D], fp32, tag="cv_stage")
                nc.sync.dma_start(cv_stage[:], cv_rows[i:i + 1, :])
                cvb = sm.tile([P, D], fp32, tag="cvb")
                nc.gpsimd.partition_broadcast(cvb[:], cv_stage[0:1, :])

                eb = st.tile([P, SO, D], f32r, tag="eb")
                nc.sync.dma_start(eb[:], ve.ap()[i].rearrange("o p d -> p o d"))

                sc = sm.tile([P, SO], fp32, tag="sc")
                for t in range(SO):
                    prod = sm.tile([P, D], fp32, tag="prod")
                    nc.vector.scalar_tensor_tensor(
                        out=prod[:],
                        in0=eb[:, t, :].bitcast(fp32), scalar=INV_SQRT,
                        in1=cvb[:],
                        op0=Alu.mult, op1=Alu.mult,
                        accum_out=sc[:, t:t + 1])

                p_all = sm.tile([P, SO], f32r, tag="p_all")
                nc.scalar.activation(p_all[:], sc[:], Act.Exp)

                # Z = sum_s p  (ones^T @ p -> [1, SO], then free-reduce)
                pz = ps_z.tile([1, SO], fp32, tag="pz")
                nc.tensor.matmul(pz[:], ones_r[:, 0:1], p_all[:],
                                 start=True, stop=True)
                z1 = sm.tile([1, 2], fp32, tag="z1")
                nc.vector.tensor_reduce(z1[:, 0:1], pz[:], Ax.X, Alu.add)
                nc.vector.reciprocal(z1[:, 1:2], z1[:, 0:1])

                # ebar_un = p^T @ e  (row accumulated in PSUM)
                prA = ps_rowA.tile([1, 512], fp32, tag="prA")
                prB = ps_rowB.tile([1, 256], fp32, tag="prB")
                for t in range(SO):
                    lhs = p_all[:, t:t + 1]
                    nc.tensor.matmul(prA[:], lhs,
                                     eb[:, t, 0:512],
                                     start=(t == 0), stop=(t == SO - 1))
                    nc.tensor.matmul(prB[:], lhs,
                                     eb[:, t, 512:768],
                                     start=(t == 0), stop=(t == SO - 1))

                # normalize by 1/Z on the way out of PSUM
                row_sb = sm.tile([1, D], fp32, tag="row_sb")
                nc.scalar.activation(row_sb[:, 0:512], prA[:], Act.Copy,
                                     scale=z1[:, 1:2])
                nc.scalar.activation(row_sb[:, 512:768], prB[:], Act.Copy,
                                     scale=z1[:, 1:2])
                nc.sync.dma_start(ebar_rows[i:i + 1, :], row_sb[:])

            # ---- epilogue on [*, 10] ----
            ebarT = cw.tile([P, 6, NBQ], fp32, tag="ebarT")
            for j in range(6):
                pt = ps_small.tile([P, NBQ], fp32, tag="ps")
                nc.tensor.transpose(pt[:], ebar_rows[:, ts(j, P)], id10[:])
                nc.scalar.copy(ebarT[:, j, :], pt[:])

            vbarT = cw.tile([P, 4, NBQ], fp32, tag="vbarT")
            for mc in range(4):
                pv = ps_small.tile([P, NBQ], fp32, tag="ps")
                for kc in range(6):
                    nc.tensor.matmul(pv[:], wvpT[:, kc, ts(mc, P)],
                                     ebarT[:, kc, :],
                                     start=(kc == 0), stop=(kc == 5))
                nc.vector.tensor_scalar(vbarT[:, mc, :], pv[:],
                                        sb_bvp[:, mc:mc + 1], None, Alu.add)

            pooledT = cw.tile([P, 4, NBQ], fp32, tag="pooledT")
            for mc in range(4):
                pv = ps_small.tile([P, NBQ], fp32, tag="ps")
                for kc in range(4):
                    nc.tensor.matmul(pv[:], wvT[:, kc, ts(mc, P)],
                                     vbarT[:, kc, :],
                                     start=(kc == 0), stop=(kc == 3))
                nc.scalar.copy(pooledT[:, mc, :], pv[:])

            xT = cw.tile([P, 4, NBQ], fp32, tag="xT")
            for mc in range(4):
                pv = ps_small.tile([P, NBQ], fp32, tag="ps")
                for kc in range(8):
                    rhs = pooledT[:, kc, :] if kc < 4 else qT[:, kc - 4, :]
                    nc.tensor.matmul(pv[:], w1T[:, kc, ts(mc, P)], rhs,
                                     start=(kc == 0), stop=(kc == 7))
                nc.scalar.activation(xT[:, mc, :], pv[:], Act.Relu,
                                     bias=sb_b1[:, mc:mc + 1])

            po = ps_small.tile([NBQ, OUT], fp32, tag="ps")
            for kc in range(4):
                nc.tensor.matmul(po[:], xT[:, kc, :], w2T[:, kc, :],
                                 start=(kc == 0), stop=(kc == 3))

            # + mask + b2, softmax over the 5 logits
            lg = sm.tile([NBQ, OUT], fp32, tag="lg")
            nc.vector.tensor_add(lg[:], po[:], sb_mb[:])
            ex = sm.tile([NBQ, OUT], fp32, tag="ex")
            nc.scalar.activation(ex[:], lg[:], Act.Exp)
            ssum = sm.tile([NBQ, 2], fp32, tag="ssum")
            nc.vector.tensor_reduce(ssum[:, 0:1], ex[:], Ax.X, Alu.add)
            nc.vector.reciprocal(ssum[:, 1:2], ssum[:, 0:1])
            res = sm.tile([NBQ, OUT], fp32, tag="res")
            nc.vector.tensor_scalar(res[:], ex[:], ssum[:, 1:2], None, Alu.mult)
            nc.sync.dma_start(out.ap(), res[:])

    nc.compile()
    return nc


def _get_compiled():
    global _COMPILED
    if _COMPILED is None:
        _COMPILED = _build()
    return _COMPILED


def _tile_lhst(w):
    """[K, M] -> [128, K//128, M] partition-tiled, contiguous."""
    K, M = w.shape
    return np.ascontiguousarray(w.reshape(K // P, P, M).transpose(1, 0, 2))


def _tile_bias(b):
    return np.ascontiguousarray(b.reshape(-1, P).T)


def make_in_maps(video_enc, ques_enc, output_mask,
                 Wvp_, bvp_, Wqp_, bqp_, Wk_, Wv_, Wq_, W1_, b1_, W2_, b2_):
    ve_all = np.ascontiguousarray(video_enc, np.float32).reshape(80, SO, P, D)
    q0 = np.ascontiguousarray(ques_enc[:, :, 0, :], np.float32).reshape(80, D)
    om = np.ascontiguousarray(output_mask, np.float32).reshape(80, OUT)

    common = dict(
        WqpT=_tile_lhst(Wqp_.T),          # [768, 512]
        WqT=_tile_lhst(Wq_.T),            # [512, 1024]
        Wk=_tile_lhst(Wk_),               # [1024, 512]
        Wvp=_tile_lhst(Wvp_),             # [512, 768]  (rhs layout)
        WvpT=_tile_lhst(Wvp_.T),          # [768, 512]
        WvT=_tile_lhst(Wv_.T),            # [512, 512]
        W1T=_tile_lhst(W1_.T),            # [1024, 512]
        W2T=_tile_lhst(W2_.T),            # [512, 5]
        bqp=_tile_bias(bqp_), bvp=_tile_bias(bvp_), b1=_tile_bias(b1_),
        b2=np.ascontiguousarray(b2_, np.float32).reshape(1, OUT),
    )
    common = {k: np.ascontiguousarray(v, np.float32) for k, v in common.items()}

    in_maps = []
    for c in range(N_CORES):
        sl = slice(c * NBQ, (c + 1) * NBQ)
        m = dict(common)
        m["ve"] = ve_all[sl]
        m["q0T"] = np.ascontiguousarray(
            q0[sl].T.reshape(6, P, NBQ).transpose(1, 0, 2))
        m["omask"] = om[sl]
        in_maps.append(m)
    return in_maps


def kernel(**inputs):
    global LAST_RESULTS
    from concourse.bass_utils import run_bass_kernel_spmd

    f = lambda k: np.asarray(inputs[k], np.float32)
    in_maps = make_in_maps(
        f("video_enc"), f("ques_enc"), f("output_mask"),
        f("Wvp"), f("bvp"), f("Wqp"), f("bqp"), f("Wk"), f("Wv"), f("Wq"),
        f("W1"), f("b1"), f("W2"), f("b2"))

    nc = _get_compiled()
    res = run_bass_kernel_spmd(nc, in_maps, core_ids=list(range(N_CORES)))
    LAST_RESULTS = res
    outs = np.concatenate([res.results[c]["out"] for c in range(N_CORES)], 0)
    return outs.reshape(16, 5, OUT).astype(np.float32)


# revision 25
# speedup vs baseline: 1.1066x; 1.1066x over previous
"""Trainium2 Bass kernel for nn_ClassifierAttn (single-query attention pooling).

Math restructuring (exact, up to float assoc):
  reference computes, per (b,q):
    v = e @ Wvp^T + bvp            [S, H]      (e = video_enc[b,q], S=2048)
    k = v @ Wk^T ; val = v @ Wv^T
    scores = k . qq / 32 ; attn = softmax(scores)
    pooled = attn @ val
  Since the attention query is a single vector per (b,q):
    scores[s] = (e[s] . cv + const)/32   with cv = Wvp^T (Wk^T qq),
  and the const term cancels in softmax. Similarly
    pooled = ((attn @ e) @ Wvp^T + bvp) @ Wv^T.
  So the big tensor e is touched by exactly two mat-vec passes:
    (1) scores = e . cv  (fused mul+reduce on VectorE)
    (2) ebar = sum_s p[s] e[s,:]  (PE matmul, p = unnormalized softmax weights)
  Everything else is tiny [10, *] linear algebra.

Sharding: (B*QPV)=80 pairs, 10 per core across 8 cores; weights replicated.
"""

import os
import numpy as np

P = 128
NBQ = 10          # (b,q) pairs per core
SO = 16           # S / P
S = 2048
D = 768
H = 512
H2 = 1024
OUT = 5
N_CORES = 8
INV_SQRT = 1.0 / 32.0   # 1/sqrt(2H)

_COMPILED = None
LAST_RESULTS = None


def ts(i, size):
    return slice(i * size, (i + 1) * size)


def _build():
    import concourse.bass as bass  # noqa: F401
    import concourse.mybir as mybir
    import concourse.tile as tile
    from concourse import bacc
    from concourse.masks import make_identity

    fp32 = mybir.dt.float32
    f32r = mybir.dt.float32r
    Alu = mybir.AluOpType
    Act = mybir.ActivationFunctionType
    Ax = mybir.AxisListType

    nc = bacc.Bacc("TRN2", target_bir_lowering=False, debug=False,
                   num_devices=N_CORES)

    # ---- DRAM I/O (all host-pre-tiled to DMA-contiguous layouts) ----
    ve = nc.dram_tensor("ve", [NBQ, P, SO, D], f32r, kind="ExternalInput")
    q0T = nc.dram_tensor("q0T", [P, 6, NBQ], fp32, kind="ExternalInput")
    omask = nc.dram_tensor("omask", [NBQ, OUT], fp32, kind="ExternalInput")
    WqpT = nc.dram_tensor("WqpT", [P, 6, H], fp32, kind="ExternalInput")
    WqT = nc.dram_tensor("WqT", [P, 4, H2], fp32, kind="ExternalInput")
    Wk = nc.dram_tensor("Wk", [P, 8, H], fp32, kind="ExternalInput")
    Wvp = nc.dram_tensor("Wvp", [P, 4, D], fp32, kind="ExternalInput")
    WvpT = nc.dram_tensor("WvpT", [P, 6, H], fp32, kind="ExternalInput")
    WvT = nc.dram_tensor("WvT", [P, 4, H], fp32, kind="ExternalInput")
    W1T = nc.dram_tensor("W1T", [P, 8, H], fp32, kind="ExternalInput")
    W2T = nc.dram_tensor("W2T", [P, 4, OUT], fp32, kind="ExternalInput")
    bqp = nc.dram_tensor("bqp", [P, 4], fp32, kind="ExternalInput")
    bvp = nc.dram_tensor("bvp", [P, 4], fp32, kind="ExternalInput")
    b1 = nc.dram_tensor("b1", [P, 4], fp32, kind="ExternalInput")
    b2 = nc.dram_tensor("b2", [1, OUT], fp32, kind="ExternalInput")
    out = nc.dram_tensor("out", [NBQ, OUT], fp32, kind="ExternalOutput")

    with tile.TileContext(nc) as tc:
        with (
            tc.tile_pool(name="const", bufs=1) as cw,
            tc.tile_pool(name="stream", bufs=2) as st,
            tc.tile_pool(name="small", bufs=2) as sm,
            tc.tile_pool(name="ps_small", bufs=2, space="PSUM") as ps_small,
            tc.tile_pool(name="ps_rowA", bufs=2, space="PSUM") as ps_rowA,
            tc.tile_pool(name="ps_rowB", bufs=2, space="PSUM") as ps_rowB,
        ):
            # ---- load weights / small inputs ----
            def mkload(pool):
                def load(dram, shape):
                    t = pool.tile(shape, fp32, tag=dram.name + "_sb")
                    nc.sync.dma_start(t[:], dram.ap())
                    return t
                return load

            load = mkload(cw)
            wvpT = load(WvpT, [P, 6, H])
            wvT = load(WvT, [P, 4, H])
            w1T = load(W1T, [P, 8, H])
            w2T = load(W2T, [P, 4, OUT])
            sb_bqp = load(bqp, [P, 4])
            sb_bvp = load(bvp, [P, 4])
            sb_b1 = load(b1, [P, 4])
            sb_q0T = load(q0T, [P, 6, NBQ])
            sb_om = load(omask, [NBQ, OUT])

            sb_b2b = cw.tile([NBQ, OUT], fp32, tag="b2b")
            nc.sync.dma_start(sb_b2b[:], b2.ap().to_broadcast((NBQ, OUT)))
            sb_mb = cw.tile([NBQ, OUT], fp32, tag="mb")
            nc.vector.tensor_add(sb_mb[:], sb_om[:], sb_b2b[:])

            ones16 = cw.tile([P, SO], fp32, tag="ones16")
            nc.gpsimd.memset(ones16[:], 1.0)
            id10 = cw.tile([NBQ, NBQ], fp32, tag="id10")
            make_identity(nc, id10[:])

            # ---- prologue: q path -> cv_rows [10, 768] ----
            # prologue-only weights live in their own pool, freed before the
            # big streaming tiles are allocated
            qT = cw.tile([P, 4, NBQ], fp32, tag="qT")
            cv_rows = cw.tile([NBQ, D], fp32, tag="cv_rows")
            with tc.tile_pool(name="prologue_w", bufs=2) as pw:
                def loadp(dram, shape):
                    t = pw.tile(shape, fp32, tag="wpro")
                    nc.sync.dma_start(t[:], dram.ap())
                    return t

                wqpT = loadp(WqpT, [P, 6, H])
                for mc in range(4):
                    pq = ps_small.tile([P, NBQ], fp32, tag="ps")
                    for kc in range(6):
                        nc.tensor.matmul(pq[:], wqpT[:, kc, ts(mc, P)],
                                         sb_q0T[:, kc, :],
                                         start=(kc == 0), stop=(kc == 5))
                    nc.vector.tensor_scalar(qT[:, mc, :], pq[:],
                                            sb_bqp[:, mc:mc + 1], None, Alu.add)

                wqT = loadp(WqT, [P, 4, H2])
                qqT = cw.tile([P, 8, NBQ], fp32, tag="qqT")
                for mc in range(8):
                    pq = ps_small.tile([P, NBQ], fp32, tag="ps")
                    for kc in range(4):
                        nc.tensor.matmul(pq[:], wqT[:, kc, ts(mc, P)],
                                         qT[:, kc, :],
                                         start=(kc == 0), stop=(kc == 3))
                    nc.scalar.copy(qqT[:, mc, :], pq[:])

                wk = loadp(Wk, [P, 8, H])
                ckT = cw.tile([P, 4, NBQ], fp32, tag="ckT")
                for mc in range(4):
                    pq = ps_small.tile([P, NBQ], fp32, tag="ps")
                    for kc in range(8):
                        nc.tensor.matmul(pq[:], wk[:, kc, ts(mc, P)],
                                         qqT[:, kc, :],
                                         start=(kc == 0), stop=(kc == 7))
                    nc.scalar.copy(ckT[:, mc, :], pq[:])

                wvp = loadp(Wvp, [P, 4, D])
                for nsl, nsz in ((slice(0, 512), 512), (slice(512, 768), 256)):
                    pq = ps_small.tile([NBQ, 512], fp32, tag="ps")
                    for kc in range(4):
                        nc.tensor.matmul(pq[:, :nsz], ckT[:, kc, :],
                                         wvp[:, kc, nsl],
                                         start=(kc == 0), stop=(kc == 3))
                    nc.scalar.copy(cv_rows[:, nsl], pq[:, :nsz])

            # ---- streaming phase over the 10 (b,q) pairs ----
            # Per 128-row tile t: fused dot (DVE) -> exp (ACT) -> two PE
            # matmuls accumulating ebar_un. A ones-column at d=768 makes the
            # second matmul also accumulate Z = sum(p), so each tile is fully
            # independent and all three engines pipeline.
            DW = 772  # 768 data + ones col at 768 + pad
            ebar_rows = cw.tile([NBQ, D], fp32, tag="ebar_rows")
            for i in range(NBQ):
                cv_stage = sm.tile([1, D], fp32, tag="cv_stage")
                nc.sync.dma_start(cv_stage[:], cv_rows[i:i + 1, :])
                cvb = sm.tile([P, D], fp32, tag="cvb")
                nc.gpsimd.partition_broadcast(cvb[:], cv_stage[0:1, :])

                eb = st.tile([P, SO, DW], f32r, tag="eb")
                nc.sync.dma_start(eb[:, :, 0:D], ve.ap()[i])
                nc.vector.tensor_copy(eb[:, :, D:D + 2],
                                      ones16[:, :, None].to_broadcast((P, SO, 2)))

                prA = ps_rowA.tile([1, 512], fp32, tag="prA")
                prB = ps_rowB.tile([1, 258], fp32, tag="prB")
                for t in range(SO):
                    prod = sm.tile([P, D], fp32, tag="prod")
                    scc = sm.tile([P, 1], fp32, tag="scc")
                    nc.vector.scalar_tensor_tensor(
                        out=prod[:],
                        in0=eb[:, t, 0:D].bitcast(fp32), scalar=INV_SQRT,
                        in1=cvb[:],
                        op0=Alu.mult, op1=Alu.mult,
                        accum_out=scc[:])
                    pcol = sm.tile([P, 1], f32r, tag="pcol")
                    nc.scalar.activation(pcol[:], scc[:], Act.Exp)
                    nc.tensor.matmul(prA[:], pcol[:],
                                     eb[:, t, 0:512],
                                     start=(t == 0), stop=(t == SO - 1))
                    nc.tensor.matmul(prB[:], pcol[:],
                                     eb[:, t, 512:D + 2],
                                     start=(t == 0), stop=(t == SO - 1))

                # Z rode along in prB col 256; normalize by 1/Z out of PSUM
                rz = sm.tile([1, 1], fp32, tag="rz")
                nc.vector.reciprocal(rz[:], prB[0:1, 256:257])
                row_sb = sm.tile([1, D], fp32, tag="row_sb")
                nc.scalar.activation(row_sb[:, 0:512], prA[:], Act.Copy,
                                     scale=rz[0:1, 0:1])
                nc.scalar.activation(row_sb[:, 512:768], prB[:, 0:256],
                                     Act.Copy, scale=rz[0:1, 0:1])
                nc.sync.dma_start(ebar_rows[i:i + 1, :], row_sb[:])

            # ---- epilogue on [*, 10] ----
            ebarT = cw.tile([P, 6, NBQ], fp32, tag="ebarT")
            for j in range(6):
                pt = ps_small.tile([P, NBQ], fp32, tag="ps")
                nc.tensor.transpose(pt[:], ebar_rows[:, ts(j, P)], id10[:])
                nc.scalar.copy(ebarT[:, j, :], pt[:])

            vbarT = cw.tile([P, 4, NBQ], fp32, tag="vbarT")
            for mc in range(4):
                pv = ps_small.tile([P, NBQ], fp32, tag="ps")
                for kc in range(6):
                    nc.tensor.matmul(pv[:], wvpT[:, kc, ts(mc, P)],
                                     ebarT[:, kc, :],
                                     start=(kc == 0), stop=(kc == 5))
                nc.vector.tensor_scalar(vbarT[:, mc, :], pv[:],
                                        sb_bvp[:, mc:mc + 1], None, Alu.add)

            pooledT = cw.tile([P, 4, NBQ], fp32, tag="pooledT")
            for mc in range(4):
                pv = ps_small.tile([P, NBQ], fp32, tag="ps")
                for kc in range(4):
                    nc.tensor.matmul(pv[:], wvT[:, kc, ts(mc, P)],
                                     vbarT[:, kc, :],
                                     start=(kc == 0), stop=(kc == 3))
                nc.scalar.copy(pooledT[:, mc, :], pv[:])

            xT = cw.tile([P, 4, NBQ], fp32, tag="xT")
            for mc in range(4):
                pv = ps_small.tile([P, NBQ], fp32, tag="ps")
                for kc in range(8):
                    rhs = pooledT[:, kc, :] if kc < 4 else qT[:, kc - 4, :]
                    nc.tensor.matmul(pv[:], w1T[:, kc, ts(mc, P)], rhs,
                                     start=(kc == 0), stop=(kc == 7))
                nc.scalar.activation(xT[:, mc, :], pv[:], Act.Relu,
                                     bias=sb_b1[:, mc:mc + 1])

            po = ps_small.tile([NBQ, OUT], fp32, tag="ps")
            for kc in range(4):
                nc.tensor.matmul(po[:], xT[:, kc, :], w2T[:, kc, :],
                                 start=(kc == 0), stop=(kc == 3))

            # + mask + b2, softmax over the 5 logits
            lg = sm.tile([NBQ, OUT], fp32, tag="lg")
            nc.vector.tensor_add(lg[:], po[:], sb_mb[:])
            ex = sm.tile([NBQ, OUT], fp32, tag="ex")
            nc.scalar.activation(ex[:], lg[:], Act.Exp)
            ssum = sm.tile([NBQ, 2], fp32, tag="ssum")
            nc.vector.tensor_reduce(ssum[:, 0:1], ex[:], Ax.X, Alu.add)
            nc.vector.reciprocal(ssum[:, 1:2], ssum[:, 0:1])
            res = sm.tile([NBQ, OUT], fp32, tag="res")
            nc.vector.tensor_scalar(res[:], ex[:], ssum[:, 1:2], None, Alu.mult)
            nc.sync.dma_start(out.ap(), res[:])

    nc.compile()
    return nc


def _get_compiled():
    global _COMPILED
    if _COMPILED is None:
        _COMPILED = _build()
    return _COMPILED


def _tile_lhst(w):
    """[K, M] -> [128, K//128, M] partition-tiled, contiguous."""
    K, M = w.shape
    return np.ascontiguousarray(w.reshape(K // P, P, M).transpose(1, 0, 2))


def _tile_bias(b):
    return np.ascontiguousarray(b.reshape(-1, P).T)


def make_in_maps(video_enc, ques_enc, output_mask,
                 Wvp_, bvp_, Wqp_, bqp_, Wk_, Wv_, Wq_, W1_, b1_, W2_, b2_):
    ve_all = np.ascontiguousarray(video_enc, np.float32).reshape(
        80, SO, P, D).transpose(0, 2, 1, 3)
    q0 = np.ascontiguousarray(ques_enc[:, :, 0, :], np.float32).reshape(80, D)
    om = np.ascontiguousarray(output_mask, np.float32).reshape(80, OUT)

    common = dict(
        WqpT=_tile_lhst(Wqp_.T),          # [768, 512]
        WqT=_tile_lhst(Wq_.T),            # [512, 1024]
        Wk=_tile_lhst(Wk_),               # [1024, 512]
        Wvp=_tile_lhst(Wvp_),             # [512, 768]  (rhs layout)
        WvpT=_tile_lhst(Wvp_.T),          # [768, 512]
        WvT=_tile_lhst(Wv_.T),            # [512, 512]
        W1T=_tile_lhst(W1_.T),            # [1024, 512]
        W2T=_tile_lhst(W2_.T),            # [512, 5]
        bqp=_tile_bias(bqp_), bvp=_tile_bias(bvp_), b1=_tile_bias(b1_),
        b2=np.ascontiguousarray(b2_, np.float32).reshape(1, OUT),
    )
    common = {k: np.ascontiguousarray(v, np.float32) for k, v in common.items()}

    in_maps = []
    for c in range(N_CORES):
        sl = slice(c * NBQ, (c + 1) * NBQ)
        m = dict(common)
        m["ve"] = np.ascontiguousarray(ve_all[sl])
        m["q0T"] = np.ascontiguousarray(
            q0[sl].T.reshape(6, P, NBQ).transpose(1, 0, 2))
        m["omask"] = om[sl]
        in_maps.append(m)
    return in_maps


def kernel(**inputs):
    global LAST_RESULTS
    from concourse.bass_utils import run_bass_kernel_spmd

    f = lambda k: np.asarray(inputs[k], np.float32)
    in_maps = make_in_maps(
        f("video_enc"), f("ques_enc"), f("output_mask"),
        f("Wvp"), f("bvp"), f("Wqp"), f("bqp"), f("Wk"), f("Wv"), f("Wq"),
        f("W1"), f("b1"), f("W2"), f("b2"))

    nc = _get_compiled()
    res = run_bass_kernel_spmd(nc, in_maps, core_ids=list(range(N_CORES)))
    LAST_RESULTS = res
    outs = np.concatenate([res.results[c]["out"] for c in range(N_CORES)], 0)
    return outs.reshape(16, 5, OUT).astype(np.float32)


# revision 28
# speedup vs baseline: 1.1356x; 1.0262x over previous
"""Trainium2 Bass kernel for nn_ClassifierAttn (single-query attention pooling).

Math restructuring (exact, up to float assoc):
  reference computes, per (b,q):
    v = e @ Wvp^T + bvp            [S, H]      (e = video_enc[b,q], S=2048)
    k = v @ Wk^T ; val = v @ Wv^T
    scores = k . qq / 32 ; attn = softmax(scores)
    pooled = attn @ val
  Since the attention query is a single vector per (b,q):
    scores[s] = (e[s] . cv + const)/32   with cv = Wvp^T (Wk^T qq),
  and the const term cancels in softmax. Similarly
    pooled = ((attn @ e) @ Wvp^T + bvp) @ Wv^T.
  So the big tensor e is touched by exactly two mat-vec passes:
    (1) scores = e . cv  (fused mul+reduce on VectorE)
    (2) ebar = sum_s p[s] e[s,:]  (PE matmul, p = unnormalized softmax weights)
  Everything else is tiny [10, *] linear algebra.

Sharding: (B*QPV)=80 pairs, 10 per core across 8 cores; weights replicated.
"""

import os
import numpy as np

P = 128
NBQ = 10          # (b,q) pairs per core
SO = 16           # S / P
S = 2048
D = 768
H = 512
H2 = 1024
OUT = 5
N_CORES = 8
INV_SQRT = 1.0 / 32.0   # 1/sqrt(2H)

_COMPILED = None
LAST_RESULTS = None


def ts(i, size):
    return slice(i * size, (i + 1) * size)


def _build():
    import concourse.bass as bass  # noqa: F401
    import concourse.mybir as mybir
    import concourse.tile as tile
    from concourse import bacc
    from concourse.masks import make_identity

    fp32 = mybir.dt.float32
    f32r = mybir.dt.float32r
    Alu = mybir.AluOpType
    Act = mybir.ActivationFunctionType
    Ax = mybir.AxisListType

    nc = bacc.Bacc("TRN2", target_bir_lowering=False, debug=False,
                   num_devices=N_CORES)

    # ---- DRAM I/O (all host-pre-tiled to DMA-contiguous layouts) ----
    ve = nc.dram_tensor("ve", [NBQ, P, SO, D], f32r, kind="ExternalInput")
    q0T = nc.dram_tensor("q0T", [P, 6, NBQ], fp32, kind="ExternalInput")
    omask = nc.dram_tensor("omask", [NBQ, OUT], fp32, kind="ExternalInput")
    WqpT = nc.dram_tensor("WqpT", [P, 6, H], fp32, kind="ExternalInput")
    WqT = nc.dram_tensor("WqT", [P, 4, H2], fp32, kind="ExternalInput")
    Wk = nc.dram_tensor("Wk", [P, 8, H], fp32, kind="ExternalInput")
    Wvp = nc.dram_tensor("Wvp", [P, 4, D], fp32, kind="ExternalInput")
    WvpT = nc.dram_tensor("WvpT", [P, 6, H], fp32, kind="ExternalInput")
    WvT = nc.dram_tensor("WvT", [P, 4, H], fp32, kind="ExternalInput")
    W1T = nc.dram_tensor("W1T", [P, 8, H], fp32, kind="ExternalInput")
    W2T = nc.dram_tensor("W2T", [P, 4, OUT], fp32, kind="ExternalInput")
    bqp = nc.dram_tensor("bqp", [P, 4], fp32, kind="ExternalInput")
    bvp = nc.dram_tensor("bvp", [P, 4], fp32, kind="ExternalInput")
    b1 = nc.dram_tensor("b1", [P, 4], fp32, kind="ExternalInput")
    b2 = nc.dram_tensor("b2", [1, OUT], fp32, kind="ExternalInput")
    out = nc.dram_tensor("out", [NBQ, OUT], fp32, kind="ExternalOutput")

    with tile.TileContext(nc) as tc:
        with (
            tc.tile_pool(name="const", bufs=1) as cw,
            tc.tile_pool(name="stream", bufs=2) as st,
            tc.tile_pool(name="small", bufs=2) as sm,
            tc.tile_pool(name="ps_small", bufs=2, space="PSUM") as ps_small,
            tc.tile_pool(name="ps_rowA", bufs=2, space="PSUM") as ps_rowA,
            tc.tile_pool(name="ps_rowB", bufs=2, space="PSUM") as ps_rowB,
            tc.tile_pool(name="ps_warm", bufs=1, space="PSUM") as ps_warm,
        ):
            # ---- load weights / small inputs ----
            def mkload(pool):
                def load(dram, shape):
                    t = pool.tile(shape, fp32, tag=dram.name + "_sb")
                    nc.sync.dma_start(t[:], dram.ap())
                    return t
                return load

            load = mkload(cw)
            wvpT = load(WvpT, [P, 6, H])
            wvT = load(WvT, [P, 4, H])
            w1T = load(W1T, [P, 8, H])
            w2T = load(W2T, [P, 4, OUT])
            sb_bqp = load(bqp, [P, 4])
            sb_bvp = load(bvp, [P, 4])
            sb_b1 = load(b1, [P, 4])
            sb_q0T = load(q0T, [P, 6, NBQ])
            sb_om = load(omask, [NBQ, OUT])

            sb_b2b = cw.tile([NBQ, OUT], fp32, tag="b2b")
            nc.sync.dma_start(sb_b2b[:], b2.ap().to_broadcast((NBQ, OUT)))
            sb_mb = cw.tile([NBQ, OUT], fp32, tag="mb")
            nc.vector.tensor_add(sb_mb[:], sb_om[:], sb_b2b[:])

            ones16 = cw.tile([P, SO], fp32, tag="ones16")
            nc.gpsimd.memset(ones16[:], 1.0)
            bf16 = mybir.dt.bfloat16
            warm_w = cw.tile([P, 2], bf16, tag="warm_w")
            nc.vector.tensor_copy(warm_w[:], ones16[:, 0:2])
            id10 = cw.tile([NBQ, NBQ], fp32, tag="id10")
            make_identity(nc, id10[:])

            # ---- prologue: q path -> cv_rows [10, 768] ----
            # prologue-only weights live in their own pool, freed before the
            # big streaming tiles are allocated
            qT = cw.tile([P, 4, NBQ], fp32, tag="qT")
            cv_rows = cw.tile([NBQ, D], fp32, tag="cv_rows")
            with tc.tile_pool(name="prologue_w", bufs=2) as pw:
                def loadp(dram, shape):
                    t = pw.tile(shape, fp32, tag="wpro")
                    nc.sync.dma_start(t[:], dram.ap())
                    return t

                wqpT = loadp(WqpT, [P, 6, H])
                for mc in range(4):
                    pq = ps_small.tile([P, NBQ], fp32, tag="ps")
                    for kc in range(6):
                        nc.tensor.matmul(pq[:], wqpT[:, kc, ts(mc, P)],
                                         sb_q0T[:, kc, :],
                                         start=(kc == 0), stop=(kc == 5))
                    nc.vector.tensor_scalar(qT[:, mc, :], pq[:],
                                            sb_bqp[:, mc:mc + 1], None, Alu.add)

                wqT = loadp(WqT, [P, 4, H2])
                qqT = cw.tile([P, 8, NBQ], fp32, tag="qqT")
                for mc in range(8):
                    pq = ps_small.tile([P, NBQ], fp32, tag="ps")
                    for kc in range(4):
                        nc.tensor.matmul(pq[:], wqT[:, kc, ts(mc, P)],
                                         qT[:, kc, :],
                                         start=(kc == 0), stop=(kc == 3))
                    nc.scalar.copy(qqT[:, mc, :], pq[:])

                wk = loadp(Wk, [P, 8, H])
                ckT = cw.tile([P, 4, NBQ], fp32, tag="ckT")
                for mc in range(4):
                    pq = ps_small.tile([P, NBQ], fp32, tag="ps")
                    for kc in range(8):
                        nc.tensor.matmul(pq[:], wk[:, kc, ts(mc, P)],
                                         qqT[:, kc, :],
                                         start=(kc == 0), stop=(kc == 7))
                    nc.scalar.copy(ckT[:, mc, :], pq[:])

                wvp = loadp(Wvp, [P, 4, D])
                for nsl, nsz in ((slice(0, 512), 512), (slice(512, 768), 256)):
                    pq = ps_small.tile([NBQ, 512], fp32, tag="ps")
                    for kc in range(4):
                        nc.tensor.matmul(pq[:, :nsz], ckT[:, kc, :],
                                         wvp[:, kc, nsl],
                                         start=(kc == 0), stop=(kc == 3))
                    nc.scalar.copy(cv_rows[:, nsl], pq[:, :nsz])

            # ---- streaming phase over the 10 (b,q) pairs ----
            # Per 128-row tile t: fused dot (DVE) -> exp (ACT) -> two PE
            # matmuls accumulating ebar_un. A ones-column at d=768 makes the
            # second matmul also accumulate Z = sum(p), so each tile is fully
            # independent and all three engines pipeline.
            DW = 772  # 768 data + ones col at 768 + pad
            ebar_rows = cw.tile([NBQ, D], fp32, tag="ebar_rows")
            for i in range(NBQ):
                cv_stage = sm.tile([1, D], fp32, tag="cv_stage")
                nc.sync.dma_start(cv_stage[:], cv_rows[i:i + 1, :])
                cvb = sm.tile([P, D], fp32, tag="cvb")
                nc.gpsimd.partition_broadcast(cvb[:], cv_stage[0:1, :])

                eb = st.tile([P, SO, DW], f32r, tag="eb")
                nc.sync.dma_start(eb[:, :, 0:D], ve.ap()[i])
                nc.vector.tensor_copy(eb[:, :, D:D + 2],
                                      ones16[:, :, None].to_broadcast((P, SO, 2)))

                prA = ps_rowA.tile([1, 512], fp32, tag="prA")
                prB = ps_rowB.tile([1, 258], fp32, tag="prB")
                pwarm = ps_warm.tile([2, 2], fp32, tag="pwarm")
                for t in range(SO):
                    # tiny bf16 matmul keeps the PE HAM clock-gate warm
                    # (f32r-mode matmuls don't count as PE activity)
                    nc.tensor.matmul(pwarm[:], warm_w[:, 0:2], warm_w[:, 0:2],
                                     start=True, stop=True,
                                     skip_group_check=True)
                    prod = sm.tile([P, D], fp32, tag="prod")
                    scc = sm.tile([P, 1], fp32, tag="scc")
                    nc.vector.scalar_tensor_tensor(
                        out=prod[:],
                        in0=eb[:, t, 0:D].bitcast(fp32), scalar=INV_SQRT,
                        in1=cvb[:],
                        op0=Alu.mult, op1=Alu.mult,
                        accum_out=scc[:])
                    pcol = sm.tile([P, 1], f32r, tag="pcol")
                    nc.scalar.activation(pcol[:], scc[:], Act.Exp)
                    nc.tensor.matmul(prA[:], pcol[:],
                                     eb[:, t, 0:512],
                                     start=(t == 0), stop=(t == SO - 1))
                    nc.tensor.matmul(prB[:], pcol[:],
                                     eb[:, t, 512:D + 2],
                                     start=(t == 0), stop=(t == SO - 1))

                # Z rode along in prB col 256; normalize by 1/Z out of PSUM
                rz = sm.tile([1, 1], fp32, tag="rz")
                nc.vector.reciprocal(rz[:], prB[0:1, 256:257])
                row_sb = sm.tile([1, D], fp32, tag="row_sb")
                nc.scalar.activation(row_sb[:, 0:512], prA[:], Act.Copy,
                                     scale=rz[0:1, 0:1])
                nc.scalar.activation(row_sb[:, 512:768], prB[:, 0:256],
                                     Act.Copy, scale=rz[0:1, 0:1])
                nc.sync.dma_start(ebar_rows[i:i + 1, :], row_sb[:])

            # ---- epilogue on [*, 10] ----
            ebarT = cw.tile([P, 6, NBQ], fp32, tag="ebarT")
            for j in range(6):
                pt = ps_small.tile([P, NBQ], fp32, tag="ps")
                nc.tensor.transpose(pt[:], ebar_rows[:, ts(j, P)], id10[:])
                nc.scalar.copy(ebarT[:, j, :], pt[:])

            vbarT = cw.tile([P, 4, NBQ], fp32, tag="vbarT")
            for mc in range(4):
                pv = ps_small.tile([P, NBQ], fp32, tag="ps")
                for kc in range(6):
                    nc.tensor.matmul(pv[:], wvpT[:, kc, ts(mc, P)],
                                     ebarT[:, kc, :],
                                     start=(kc == 0), stop=(kc == 5))
                nc.vector.tensor_scalar(vbarT[:, mc, :], pv[:],
                                        sb_bvp[:, mc:mc + 1], None, Alu.add)

            pooledT = cw.tile([P, 4, NBQ], fp32, tag="pooledT")
            for mc in range(4):
                pv = ps_small.tile([P, NBQ], fp32, tag="ps")
                for kc in range(4):
                    nc.tensor.matmul(pv[:], wvT[:, kc, ts(mc, P)],
                                     vbarT[:, kc, :],
                                     start=(kc == 0), stop=(kc == 3))
                nc.scalar.copy(pooledT[:, mc, :], pv[:])

            xT = cw.tile([P, 4, NBQ], fp32, tag="xT")
            for mc in range(4):
                pv = ps_small.tile([P, NBQ], fp32, tag="ps")
                for kc in range(8):
                    rhs = pooledT[:, kc, :] if kc < 4 else qT[:, kc - 4, :]
                    nc.tensor.matmul(pv[:], w1T[:, kc, ts(mc, P)], rhs,
                                     start=(kc == 0), stop=(kc == 7))
                nc.scalar.activation(xT[:, mc, :], pv[:], Act.Relu,
                                     bias=sb_b1[:, mc:mc + 1])

            po = ps_small.tile([NBQ, OUT], fp32, tag="ps")
            for kc in range(4):
                nc.tensor.matmul(po[:], xT[:, kc, :], w2T[:, kc, :],
                                 start=(kc == 0), stop=(kc == 3))

            # + mask + b2, softmax over the 5 logits
            lg = sm.tile([NBQ, OUT], fp32, tag="lg")
            nc.vector.tensor_add(lg[:], po[:], sb_mb[:])
            ex = sm.tile([NBQ, OUT], fp32, tag="ex")
            nc.scalar.activation(ex[:], lg[:], Act.Exp)
            ssum = sm.tile([NBQ, 2], fp32, tag="ssum")
            nc.vector.tensor_reduce(ssum[:, 0:1], ex[:], Ax.X, Alu.add)
            nc.vector.reciprocal(ssum[:, 1:2], ssum[:, 0:1])
            res = sm.tile([NBQ, OUT], fp32, tag="res")
            nc.vector.tensor_scalar(res[:], ex[:], ssum[:, 1:2], None, Alu.mult)
            nc.sync.dma_start(out.ap(), res[:])

    nc.compile()
    return nc


def _get_compiled():
    global _COMPILED
    if _COMPILED is None:
        _COMPILED = _build()
    return _COMPILED


def _tile_lhst(w):
    """[K, M] -> [128, K//128, M] partition-tiled, contiguous."""
    K, M = w.shape
    return np.ascontiguousarray(w.reshape(K // P, P, M).transpose(1, 0, 2))


def _tile_bias(b):
    return np.ascontiguousarray(b.reshape(-1, P).T)


def make_in_maps(video_enc, ques_enc, output_mask,
                 Wvp_, bvp_, Wqp_, bqp_, Wk_, Wv_, Wq_, W1_, b1_, W2_, b2_):
    ve_all = np.ascontiguousarray(video_enc, np.float32).reshape(
        80, SO, P, D).transpose(0, 2, 1, 3)
    q0 = np.ascontiguousarray(ques_enc[:, :, 0, :], np.float32).reshape(80, D)
    om = np.ascontiguousarray(output_mask, np.float32).reshape(80, OUT)

    common = dict(
        WqpT=_tile_lhst(Wqp_.T),          # [768, 512]
        WqT=_tile_lhst(Wq_.T),            # [512, 1024]
        Wk=_tile_lhst(Wk_),               # [1024, 512]
        Wvp=_tile_lhst(Wvp_),             # [512, 768]  (rhs layout)
        WvpT=_tile_lhst(Wvp_.T),          # [768, 512]
        WvT=_tile_lhst(Wv_.T),            # [512, 512]
        W1T=_tile_lhst(W1_.T),            # [1024, 512]
        W2T=_tile_lhst(W2_.T),            # [512, 5]
        bqp=_tile_bias(bqp_), bvp=_tile_bias(bvp_), b1=_tile_bias(b1_),
        b2=np.ascontiguousarray(b2_, np.float32).reshape(1, OUT),
    )
    common = {k: np.ascontiguousarray(v, np.float32) for k, v in common.items()}

    in_maps = []
    for c in range(N_CORES):
        sl = slice(c * NBQ, (c + 1) * NBQ)
        m = dict(common)
        m["ve"] = np.ascontiguousarray(ve_all[sl])
        m["q0T"] = np.ascontiguousarray(
            q0[sl].T.reshape(6, P, NBQ).transpose(1, 0, 2))
        m["omask"] = om[sl]
        in_maps.append(m)
    return in_maps


def kernel(**inputs):
    global LAST_RESULTS
    from concourse.bass_utils import run_bass_kernel_spmd

    f = lambda k: np.asarray(inputs[k], np.float32)
    in_maps = make_in_maps(
        f("video_enc"), f("ques_enc"), f("output_mask"),
        f("Wvp"), f("bvp"), f("Wqp"), f("bqp"), f("Wk"), f("Wv"), f("Wq"),
        f("W1"), f("b1"), f("W2"), f("b2"))

    nc = _get_compiled()
    res = run_bass_kernel_spmd(nc, in_maps, core_ids=list(range(N_CORES)))
    LAST_RESULTS = res
    outs = np.concatenate([res.results[c]["out"] for c in range(N_CORES)], 0)
    return outs.reshape(16, 5, OUT).astype(np.float32)
